# revision 1
# baseline (speedup 1.0000x reference)
"""Tensor-parallel fused attention block (QKV proj + MHA + out proj) for 8 TRN2 cores.

Sharding: 16 heads -> 2 heads per core. W1 rows (q/k/v of the core's heads) and
W2 columns are sharded; x is replicated. Each core computes a partial output
[B*T, E] (its heads' contribution through W2); the host sums the 8 partials.

Layouts (host-prepped, all bf16):
  xT   [E, B*T]   x transposed, feature-major (rhs/lhsT tiles for QKV matmuls)
  wqT  [E, 256]   w1-q rows for the core's 2 heads, transposed
  wkT  [E, 256]
  wvT  [E, 256]
  w2sT [256, E]   w2 columns for the core's heads, transposed

Per-core dataflow per batch b (T=2048 tokens):
  qT/kT [256, T] feature-major (PE: lhsT=w*T tile, rhs=xT tile)
  v     [T, 256] token-major   (PE: lhsT=xT tile, rhs=wvT tile)
  per head: ST[kt,qt] = kT_h.T @ qT_h ; PT = exp(scale*ST) (ACT, f32 psum -> bf16)
            OT[d,qt] += v_h.T-tile @ PT (PE accum) ; rowsum[1,qt] += ones.T @ PT
            OT /= rowsum broadcast via K=1 PE matmul + DVE mul -> bf16
  partial[n,o] = OT-stack.T @ w2sT (PE) -> psum -> DMA to DRAM f32
"""

import json
import types

import ml_dtypes
import numpy as np

B, T, E = 4, 2048, 2048
NH, D = 16, 128
NCORES = 8
HPC = NH // NCORES          # heads per core = 2
F = HPC * D                 # per-core qkv feature dim = 256
N = B * T                   # 8192 tokens
ET = E // 128               # 16 e-tiles
SCALE = float(1.0 / np.sqrt(D))

BF16 = ml_dtypes.bfloat16


def _split_multi_waits(m: dict) -> dict:
    """This container's walrus rejects any instruction carrying >1 semaphore
    wait; hoist extra waits into standalone single-wait EventSemaphore insts
    emitted just before, on the same engine (program order preserves semantics)."""
    for fn in m["functions"]:
        for b in fn["blocks"]:
            new_insts = []
            for i in b["instructions"]:
                si = i.get("sync_info")
                waits = (si or {}).get("on_wait") or []
                if len(waits) > 1:
                    for k, w in enumerate(waits[:-1]):
                        new_insts.append({
                            "name": f"{i['name']}-presplitwait-{k}",
                            "opcode": "EventSemaphore",
                            "engine": i["engine"],
                            "ins": [], "outs": [],
                            "sync_info": {"on_wait": [w], "on_update": []},
                        })
                    si["on_wait"] = [waits[-1]]
                new_insts.append(i)
            b["instructions"] = new_insts
    return m


def _patch_serializer(nc):
    orig = nc.to_json_bytes

    def to_json_bytes(self):
        return json.dumps(_split_multi_waits(json.loads(orig()))).encode()

    nc.to_json_bytes = types.MethodType(to_json_bytes, nc)


def build_nc(loop: int | None = None, ablate: set | None = None):
    import contextlib

    import concourse.bass as bass
    import concourse.mybir as mybir
    import concourse.tile as tile

    dt = mybir.dt
    AF = mybir.ActivationFunctionType

    nc = bass.Bass("TRN2", target_bir_lowering=False, debug=False)

    xT = nc.dram_tensor("xT", [N // 512, 128, ET, 512], dt.bfloat16,
                        kind="ExternalInput")
    wqT = nc.dram_tensor("wqT", [E, F], dt.bfloat16, kind="ExternalInput")
    wkT = nc.dram_tensor("wkT", [E, F], dt.bfloat16, kind="ExternalInput")
    wvT = nc.dram_tensor("wvT", [E, F], dt.bfloat16, kind="ExternalInput")
    w2sT = nc.dram_tensor("w2sT", [F, E], dt.bfloat16, kind="ExternalInput")
    out = nc.dram_tensor("out", [N, E], dt.bfloat16, kind="ExternalOutput")

    wq_r = wqT.rearrange("(n p) f -> p n f", p=128)     # [128, 16, 256]
    wk_r = wkT.rearrange("(n p) f -> p n f", p=128)
    wv_r = wvT.rearrange("(n p) f -> p n f", p=128)
    w2_r = w2sT.rearrange("(n p) f -> p n f", p=128)    # [128, 2, 2048]

    with tile.TileContext(nc) as tc:
        with (
            tc.tile_pool(name="wpool", bufs=1) as wpool,
            tc.tile_pool(name="xpool", bufs=4) as xpool,
            tc.tile_pool(name="qkv", bufs=2) as qkvpool,
            tc.tile_pool(name="ptpool", bufs=8) as ptpool,
            tc.tile_pool(name="otpool", bufs=2) as otpool,
            tc.tile_pool(name="opool", bufs=4) as opool,
            tc.tile_pool(name="small", bufs=2) as small,
            tc.tile_pool(name="ps_mm", bufs=3, space="PSUM") as ps_mm,
            tc.tile_pool(name="ps_acc", bufs=3, space="PSUM") as ps_acc,
            tc.tile_pool(name="ps_rs", bufs=2, space="PSUM") as ps_rs,
        ):
            # weights + constants (loaded once)
            wq_sb = wpool.tile([128, ET, F], dt.bfloat16)
            wk_sb = wpool.tile([128, ET, F], dt.bfloat16)
            wv_sb = wpool.tile([128, ET, F], dt.bfloat16)
            w2_sb = wpool.tile([128, 2, E], dt.bfloat16)
            for g in range(4):  # split across queues; finer first-matmul deps
                sl = slice(g * 4, (g + 1) * 4)
                nc.sync.dma_start(out=wq_sb[:, sl, :], in_=wq_r[:, sl, :])
                nc.sync.dma_start(out=wk_sb[:, sl, :], in_=wk_r[:, sl, :])
                nc.sync.dma_start(out=wv_sb[:, sl, :], in_=wv_r[:, sl, :])
                nc.sync.dma_start(
                    out=w2_sb[:, :, g * 512:(g + 1) * 512],
                    in_=w2_r[:, :, g * 512:(g + 1) * 512])
            # rowsum lhsT: col 0 ones, cols 1-31 zero -> each col-packed
            # matmul writes a full 32-row psum block (sum row + zeros)
            ones_pad = wpool.tile([128, 32], dt.bfloat16)
            nc.vector.memset(ones_pad, 0.0)
            nc.vector.memset(ones_pad[:, 0:1], 1.0)
            ones_f32 = wpool.tile([128, 1], dt.float32)    # final reduce lhsT
            nc.vector.memset(ones_f32, 1.0)
            ones_row = wpool.tile([1, 128], dt.float32)    # bcast lhsT (K=1)
            nc.vector.memset(ones_row, 1.0)

            # loop=K repeats the whole computation K times (HW-timing slope
            # measurement: kernel_ns = (t(K2)-t(K1))/(K2-K1)); None = run once.
            loop_cm = tc.For_i(0, loop, 1) if loop else contextlib.nullcontext()
            with loop_cm:
                _emit_body(nc, tc, dt, AF, locals(), ablate or set())
    _patch_serializer(nc)
    return nc


def _emit_body(nc, tc, dt, AF, env, ablate=frozenset()):
    xT_r, out = env["xT"], env["out"]
    wq_sb, wk_sb, wv_sb, w2_sb = env["wq_sb"], env["wk_sb"], env["wv_sb"], env["w2_sb"]
    ones_pad, ones_f32, ones_row = (env["ones_pad"], env["ones_f32"],
                                    env["ones_row"])
    xpool, qkvpool, ptpool, otpool, opool, small = (
        env["xpool"], env["qkvpool"], env["ptpool"], env["otpool"],
        env["opool"], env["small"])
    ps_mm, ps_acc, ps_rs = env["ps_mm"], env["ps_acc"], env["ps_rs"]
    if True:
            for b in range(B):
                # ---- QKV projection for batch b ----
                qT_sb = qkvpool.tile([128, HPC, T], dt.bfloat16, tag="qT")
                kT_sb = qkvpool.tile([128, HPC, T], dt.bfloat16, tag="kT")
                v_sb = qkvpool.tile([128, T // 128, F], dt.bfloat16, tag="v")
                for c in range(T // 512):  # 4 chunks of 512 tokens
                    x_tile = xpool.tile([128, ET, 512], dt.bfloat16, tag="x")
                    ci = b * 4 + c
                    # contiguous chunk tile; split across 4 DMA queues
                    for g in range(1 if "xdma" in ablate else 4):
                        nc.sync.dma_start(
                            out=x_tile[:, g * 4:(g + 1) * 4, :],
                            in_=xT_r[ci, :, g * 4:(g + 1) * 4, :],
                        )
                    for wi, (w_sb, dst) in enumerate(
                            ((wq_sb, qT_sb), (wk_sb, kT_sb))):
                        for h in range(HPC):
                            ps = ps_mm.tile([128, 512], dt.float32, tag="mm")
                            for e in range(ET):
                                nc.tensor.matmul(
                                    ps,
                                    lhsT=w_sb[:, e, h * 128:(h + 1) * 128],
                                    rhs=x_tile[:, e, :],
                                    start=(e == 0), stop=(e == ET - 1),
                                )
                            cp = nc.vector.tensor_copy if wi == 0 else nc.scalar.copy
                            cp(out=dst[:, h, c * 512:(c + 1) * 512], in_=ps)
                    for nn in range(4):  # token tiles of 128 within chunk
                        ps = ps_mm.tile([128, F], dt.float32, tag="mm")
                        for e in range(ET):
                            nc.tensor.matmul(
                                ps,
                                lhsT=x_tile[:, e, nn * 128:(nn + 1) * 128],
                                rhs=wv_sb[:, e, :],
                                start=(e == 0), stop=(e == ET - 1),
                            )
                        nc.scalar.copy(out=v_sb[:, c * 4 + nn, :], in_=ps)

                # ---- attention for batch b ----
                ot_sb = otpool.tile([128, HPC, T], dt.bfloat16, tag="ot")
                for h in range(HPC):
                    for c in range(T // 512):  # qt chunks
                        ot_ps = ps_acc.tile([128, 512], dt.float32, tag="acc")
                        rs_ps = ps_rs.tile([128, 512], dt.float32, tag="rs")
                        qs = qT_sb[:, h, c * 512:(c + 1) * 512]
                        pts = []
                        for kt in range(T // 128):  # 16 key tiles
                            st_ps = ps_mm.tile([128, 512], dt.float32, tag="mm")
                            nc.tensor.matmul(
                                st_ps,
                                lhsT=kT_sb[:, h, kt * 128:(kt + 1) * 128],
                                rhs=qs, start=True, stop=True,
                            )
                            pt = ptpool.tile([128, 512], dt.bfloat16, tag="pt")
                            if "dve_exp" in ablate:
                                nc.vector.tensor_copy(out=pt, in_=st_ps)
                            else:
                                nc.scalar.activation(
                                    out=pt, in_=st_ps, func=AF.Exp, scale=SCALE)
                            nc.tensor.matmul(
                                ot_ps,
                                lhsT=v_sb[:, kt, h * 128:(h + 1) * 128],
                                rhs=pt, start=(kt == 0), stop=(kt == T // 128 - 1),
                                skip_group_check=True,
                            )
                            pts.append(pt)
                            if kt % 4 == 3 and "ones" not in ablate:
                                # 4 back-to-back M=32 rowsum matmuls col-packed
                                # into distinct 32-col groups -> run concurrent
                                for g in range(4):
                                    nc.tensor.matmul(
                                        rs_ps[32 * g:32 * (g + 1), :],
                                        lhsT=ones_pad, rhs=pts[g],
                                        start=(kt < 4), stop=(kt >= 12),
                                        tile_position=(0, 32 * g),
                                        skip_group_check=True,
                                    )
                                pts = []
                        if "ones" in ablate:
                            nc.vector.tensor_copy(
                                out=ot_sb[:, h, c * 512:(c + 1) * 512], in_=ot_ps)
                            continue
                        # rows 0/32/64/96 of rs_ps hold group sums (rest zero):
                        # copy out, reduce all 128 partitions with one matmul
                        rs_sb = small.tile([128, 512], dt.float32, tag="rs_sb")
                        nc.vector.tensor_copy(out=rs_sb, in_=rs_ps)
                        rsum_ps = ps_rs.tile([1, 512], dt.float32, tag="rs")
                        nc.tensor.matmul(
                            rsum_ps, lhsT=ones_f32, rhs=rs_sb,
                            start=True, stop=True)
                        recip = small.tile([1, 512], dt.float32, tag="recip")
                        nc.vector.reciprocal(out=recip, in_=rsum_ps)
                        bc_ps = ps_rs.tile([128, 512], dt.float32, tag="rs")
                        nc.tensor.matmul(
                            bc_ps, lhsT=ones_row, rhs=recip,
                            start=True, stop=True)
                        bc_sb = small.tile([128, 512], dt.float32, tag="bcs")
                        nc.vector.tensor_copy(out=bc_sb, in_=bc_ps)
                        nc.vector.tensor_mul(
                            out=ot_sb[:, h, c * 512:(c + 1) * 512],
                            in0=ot_ps, in1=bc_sb)

                # ---- output projection (partial) for batch b ----
                for nn in range(T // 128):  # 16 token tiles
                    o_sb = opool.tile([128, E], dt.bfloat16, tag="o")
                    for oc in range(E // 512):  # 4 output chunks
                        ps = ps_mm.tile([128, 512], dt.float32, tag="mm")
                        for j in range(HPC):
                            nc.tensor.matmul(
                                ps,
                                lhsT=ot_sb[:, j, nn * 128:(nn + 1) * 128],
                                rhs=w2_sb[:, j, oc * 512:(oc + 1) * 512],
                                start=(j == 0), stop=(j == HPC - 1),
                            )
                        if nn % 2 == 0:
                            nc.vector.tensor_copy(
                                out=o_sb[:, oc * 512:(oc + 1) * 512], in_=ps)
                        else:
                            nc.scalar.copy(
                                out=o_sb[:, oc * 512:(oc + 1) * 512], in_=ps)
                    if "outdma" not in ablate:
                        nc.sync.dma_start(
                            out=out[b * T + nn * 128:b * T + (nn + 1) * 128, :],
                            in_=o_sb)


def prep_inputs(x: np.ndarray, w1: np.ndarray, w2: np.ndarray):
    """Host-side shard + transpose + bf16 cast. Returns in_maps for 8 cores."""
    # pretile x: chunk ci of 512 tokens -> [128 part, 16 e-tiles, 512 tok]
    xb = x.reshape(N // 512, 512, ET, 128).transpose(0, 3, 2, 1)
    xf = np.ascontiguousarray(xb).astype(BF16)
    w1r = w1.reshape(3, NH, D, E)
    in_maps = []
    for m in range(NCORES):
        hs = slice(HPC * m, HPC * (m + 1))
        wq = np.ascontiguousarray(w1r[0, hs].reshape(F, E).T).astype(BF16)
        wk = np.ascontiguousarray(w1r[1, hs].reshape(F, E).T).astype(BF16)
        wv = np.ascontiguousarray(w1r[2, hs].reshape(F, E).T).astype(BF16)
        w2s = np.ascontiguousarray(w2[:, F * m:F * (m + 1)].T).astype(BF16)
        in_maps.append({"xT": xf, "wqT": wq, "wkT": wk, "wvT": wv, "w2sT": w2s})
    return in_maps


def run(x, w1, w2, trace=False):
    from concourse import bass_utils

    nc = build_nc()
    in_maps = prep_inputs(np.asarray(x), np.asarray(w1), np.asarray(w2))
    res = bass_utils.run_bass_kernel_spmd(
        nc, in_maps, core_ids=list(range(NCORES)), trace=trace)
    acc = np.zeros((N, E), np.float32)
    for r in res.results:
        acc += r["out"]
    return acc.reshape(B, T, E), res


def kernel(x, w1, w2):
    out, _ = run(x, w1, w2, trace=False)
    return out



# revision 2
# speedup vs baseline: 1.9345x; 1.9345x over previous
"""Tensor-parallel fused attention block (QKV proj + MHA + out proj) for 8 TRN2 cores.

Sharding: 16 heads -> 2 heads per core. W1 rows (q/k/v of the core's heads) and
W2 columns are sharded; x is replicated. Each core computes a partial output
[B*T, E] (its heads' contribution through W2); the host sums the 8 partials.

Layouts (host-prepped, all bf16):
  xT   [E, B*T]   x transposed, feature-major (rhs/lhsT tiles for QKV matmuls)
  wqT  [E, 256]   w1-q rows for the core's 2 heads, transposed
  wkT  [E, 256]
  wvT  [E, 256]
  w2sT [256, E]   w2 columns for the core's heads, transposed

Per-core dataflow per batch b (T=2048 tokens):
  qT/kT [256, T] feature-major (PE: lhsT=w*T tile, rhs=xT tile)
  v     [T, 256] token-major   (PE: lhsT=xT tile, rhs=wvT tile)
  per head: ST[kt,qt] = kT_h.T @ qT_h ; PT = exp(scale*ST) (ACT, f32 psum -> bf16)
            OT[d,qt] += v_h.T-tile @ PT (PE accum)
            RS[*,qt] += ones.T @ PT (PE accum, replicated over all 128 rows)
            ot = OT * recip(RS) (DVE recip + DVE mul; no reduce/bcast chain)
  partial[n,o] = OT-stack.T @ w2sT (PE) -> psum -> DMA to DRAM bf16
    (out DMA rides the Activation HWDGE queue so it never blocks x loads
     on the SP queue)
"""

import json
import types

import ml_dtypes
import numpy as np

B, T, E = 4, 2048, 2048
NH, D = 16, 128
NCORES = 8
HPC = NH // NCORES          # heads per core = 2
F = HPC * D                 # per-core qkv feature dim = 256
N = B * T                   # 8192 tokens
ET = E // 128               # 16 e-tiles
SCALE = float(1.0 / np.sqrt(D))

BF16 = ml_dtypes.bfloat16


def _split_multi_waits(m: dict) -> dict:
    """This container's walrus rejects any instruction carrying >1 semaphore
    wait; hoist extra waits into standalone single-wait EventSemaphore insts
    emitted just before, on the same engine (program order preserves semantics)."""
    for fn in m["functions"]:
        for b in fn["blocks"]:
            new_insts = []
            for i in b["instructions"]:
                si = i.get("sync_info")
                waits = (si or {}).get("on_wait") or []
                if len(waits) > 1:
                    for k, w in enumerate(waits[:-1]):
                        new_insts.append({
                            "name": f"{i['name']}-presplitwait-{k}",
                            "opcode": "EventSemaphore",
                            "engine": i["engine"],
                            "ins": [], "outs": [],
                            "sync_info": {"on_wait": [w], "on_update": []},
                        })
                    si["on_wait"] = [waits[-1]]
                new_insts.append(i)
            b["instructions"] = new_insts
    return m


def _patch_serializer(nc):
    orig = nc.to_json_bytes

    def to_json_bytes(self):
        return json.dumps(_split_multi_waits(json.loads(orig()))).encode()

    nc.to_json_bytes = types.MethodType(to_json_bytes, nc)


def build_nc(loop: int | None = None, ablate: set | None = None):
    import contextlib

    import concourse.bass as bass
    import concourse.mybir as mybir
    import concourse.tile as tile

    dt = mybir.dt
    AF = mybir.ActivationFunctionType

    nc = bass.Bass("TRN2", target_bir_lowering=False, debug=False)

    xT = nc.dram_tensor("xT", [N // 512, 128, ET, 512], dt.bfloat16,
                        kind="ExternalInput")
    wqT = nc.dram_tensor("wqT", [E, F], dt.bfloat16, kind="ExternalInput")
    wkT = nc.dram_tensor("wkT", [E, F], dt.bfloat16, kind="ExternalInput")
    wvT = nc.dram_tensor("wvT", [E, F], dt.bfloat16, kind="ExternalInput")
    w2sT = nc.dram_tensor("w2sT", [F, E], dt.bfloat16, kind="ExternalInput")
    out = nc.dram_tensor("out", [N, E], dt.bfloat16, kind="ExternalOutput")

    wq_r = wqT.rearrange("(n p) f -> p n f", p=128)     # [128, 16, 256]
    wk_r = wkT.rearrange("(n p) f -> p n f", p=128)
    wv_r = wvT.rearrange("(n p) f -> p n f", p=128)
    w2_r = w2sT.rearrange("(n p) f -> p n f", p=128)    # [128, 2, 2048]

    with tile.TileContext(nc) as tc:
        with (
            tc.tile_pool(name="wpool", bufs=1) as wpool,
            tc.tile_pool(name="xpool", bufs=4) as xpool,
            tc.tile_pool(name="qkv", bufs=2) as qkvpool,
            tc.tile_pool(name="ptpool", bufs=8) as ptpool,
            tc.tile_pool(name="otpool", bufs=2) as otpool,
            tc.tile_pool(name="opool", bufs=4) as opool,
            tc.tile_pool(name="small", bufs=2) as small,
            tc.tile_pool(name="ps_mm", bufs=3, space="PSUM") as ps_mm,
            tc.tile_pool(name="ps_acc", bufs=3, space="PSUM") as ps_acc,
            tc.tile_pool(name="ps_rs", bufs=2, space="PSUM") as ps_rs,
        ):
            # weights + constants (loaded once)
            wq_sb = wpool.tile([128, ET, F], dt.bfloat16)
            wk_sb = wpool.tile([128, ET, F], dt.bfloat16)
            wv_sb = wpool.tile([128, ET, F], dt.bfloat16)
            w2_sb = wpool.tile([128, 2, E], dt.bfloat16)
            for g in range(4):  # split across queues; finer first-matmul deps
                sl = slice(g * 4, (g + 1) * 4)
                nc.sync.dma_start(out=wq_sb[:, sl, :], in_=wq_r[:, sl, :])
                nc.sync.dma_start(out=wk_sb[:, sl, :], in_=wk_r[:, sl, :])
                nc.sync.dma_start(out=wv_sb[:, sl, :], in_=wv_r[:, sl, :])
                nc.scalar.dma_start(
                    out=w2_sb[:, :, g * 512:(g + 1) * 512],
                    in_=w2_r[:, :, g * 512:(g + 1) * 512])
            # rowsum lhsT: all-ones [128,128] -> accumulated RS matmul leaves
            # the full column-sum REPLICATED on every output partition, so
            # normalization is just recip+mul (no reduce/bcast chain).
            ones_full = wpool.tile([128, 128], dt.bfloat16)
            nc.vector.memset(ones_full, 1.0)
            # packed-rowsum alternative (normP): col 0 ones, cols 1-31 zero
            ones_pad = wpool.tile([128, 32], dt.bfloat16)
            nc.vector.memset(ones_pad, 0.0)
            nc.vector.memset(ones_pad[:, 0:1], 1.0)
            ones_f32 = wpool.tile([128, 1], dt.float32)    # final reduce lhsT
            nc.vector.memset(ones_f32, 1.0)
            ones_row = wpool.tile([1, 128], dt.float32)    # bcast lhsT (K=1)
            nc.vector.memset(ones_row, 1.0)

            # loop=K repeats the whole computation K times (HW-timing slope
            # measurement: kernel_ns = (t(K2)-t(K1))/(K2-K1)); None = run once.
            loop_cm = tc.For_i(0, loop, 1) if loop else contextlib.nullcontext()
            with loop_cm:
                _emit_body(nc, tc, dt, AF, locals(), ablate or set())
    _patch_serializer(nc)
    return nc


def _emit_body(nc, tc, dt, AF, env, ablate=frozenset()):
    xT_r, out = env["xT"], env["out"]
    wq_sb, wk_sb, wv_sb, w2_sb = env["wq_sb"], env["wk_sb"], env["wv_sb"], env["w2_sb"]
    ones_full, ones_pad, ones_f32, ones_row = (
        env["ones_full"], env["ones_pad"], env["ones_f32"], env["ones_row"])
    xpool, qkvpool, ptpool, otpool, opool, small = (
        env["xpool"], env["qkvpool"], env["ptpool"], env["otpool"],
        env["opool"], env["small"])
    ps_mm, ps_acc, ps_rs = env["ps_mm"], env["ps_acc"], env["ps_rs"]
    odma = nc.sync if "odma_sync" in ablate else (
        nc.gpsimd if "odma_pool" in ablate else nc.scalar)
    if True:
            for b in range(B):
                # ---- QKV projection for batch b ----
                qT_sb = qkvpool.tile([128, HPC, T], dt.bfloat16, tag="qT")
                kT_sb = qkvpool.tile([128, HPC, T], dt.bfloat16, tag="kT")
                v_sb = qkvpool.tile([128, T // 128, F], dt.bfloat16, tag="v")
                for c in range(T // 512):  # 4 chunks of 512 tokens
                    x_tile = xpool.tile([128, ET, 512], dt.bfloat16, tag="x")
                    ci = b * 4 + c
                    # contiguous chunk tile; split across 4 DMA queues
                    for g in range(1 if "xdma" in ablate else 4):
                        nc.sync.dma_start(
                            out=x_tile[:, g * 4:(g + 1) * 4, :],
                            in_=xT_r[ci, :, g * 4:(g + 1) * 4, :],
                        )
                    for wi, (w_sb, dst) in enumerate(
                            ((wq_sb, qT_sb), (wk_sb, kT_sb))):
                        for h in range(HPC):
                            ps = ps_mm.tile([128, 512], dt.float32, tag="mm")
                            for e in range(ET):
                                nc.tensor.matmul(
                                    ps,
                                    lhsT=w_sb[:, e, h * 128:(h + 1) * 128],
                                    rhs=x_tile[:, e, :],
                                    start=(e == 0), stop=(e == ET - 1),
                                )
                            cp = nc.vector.tensor_copy if wi == 0 else nc.scalar.copy
                            cp(out=dst[:, h, c * 512:(c + 1) * 512], in_=ps)
                    for nn in range(4):  # token tiles of 128 within chunk
                        ps = ps_mm.tile([128, F], dt.float32, tag="mm")
                        for e in range(ET):
                            nc.tensor.matmul(
                                ps,
                                lhsT=x_tile[:, e, nn * 128:(nn + 1) * 128],
                                rhs=wv_sb[:, e, :],
                                start=(e == 0), stop=(e == ET - 1),
                            )
                        nc.scalar.copy(out=v_sb[:, c * 4 + nn, :], in_=ps)

                # ---- attention for batch b ----
                ot_sb = otpool.tile([128, HPC, T], dt.bfloat16, tag="ot")
                for h in range(HPC):
                    for c in range(T // 512):  # qt chunks
                        ot_ps = ps_acc.tile([128, 512], dt.float32, tag="acc")
                        rs_ps = ps_rs.tile([128, 512], dt.float32, tag="rs")
                        qs = qT_sb[:, h, c * 512:(c + 1) * 512]
                        pts = []
                        for kt in range(T // 128):  # 16 key tiles
                            st_ps = ps_mm.tile([128, 512], dt.float32, tag="mm")
                            nc.tensor.matmul(
                                st_ps,
                                lhsT=kT_sb[:, h, kt * 128:(kt + 1) * 128],
                                rhs=qs, start=True, stop=True,
                            )
                            pt = ptpool.tile([128, 512], dt.bfloat16, tag="pt")
                            if "dve_exp" in ablate:
                                nc.vector.tensor_copy(out=pt, in_=st_ps)
                            else:
                                nc.scalar.activation(
                                    out=pt, in_=st_ps, func=AF.Exp, scale=SCALE)
                            nc.tensor.matmul(
                                ot_ps,
                                lhsT=v_sb[:, kt, h * 128:(h + 1) * 128],
                                rhs=pt, start=(kt == 0), stop=(kt == T // 128 - 1),
                                skip_group_check=True,
                            )
                            if "ones" in ablate or "norm" in ablate:
                                pass
                            elif "normP" not in ablate:
                                # replicated rowsum: every partition = colsum
                                nc.tensor.matmul(
                                    rs_ps, lhsT=ones_full, rhs=pt,
                                    start=(kt == 0), stop=(kt == T // 128 - 1),
                                    skip_group_check=True,
                                )
                            else:
                                pts.append(pt)
                                if kt % 4 == 3:
                                    # 4 back-to-back M=32 rowsum matmuls
                                    # col-packed into distinct 32-col groups
                                    for g in range(4):
                                        nc.tensor.matmul(
                                            rs_ps[32 * g:32 * (g + 1), :],
                                            lhsT=ones_pad, rhs=pts[g],
                                            start=(kt < 4), stop=(kt >= 12),
                                            tile_position=(0, 32 * g),
                                            skip_group_check=True,
                                        )
                                    pts = []
                        if "ones" in ablate or "norm" in ablate:
                            nc.vector.tensor_copy(
                                out=ot_sb[:, h, c * 512:(c + 1) * 512], in_=ot_ps)
                            continue
                        if "normP" not in ablate:
                            recip = small.tile([128, 512], dt.float32, tag="recip")
                            nc.vector.reciprocal(out=recip, in_=rs_ps)
                            nc.vector.tensor_mul(
                                out=ot_sb[:, h, c * 512:(c + 1) * 512],
                                in0=ot_ps, in1=recip)
                            continue
                        # normP: rows 0/32/64/96 of rs_ps hold group sums:
                        # copy out (bf16), then one ones_full matmul both
                        # reduces and replicates across all 128 partitions
                        rs_sb = small.tile([128, 512], dt.bfloat16, tag="rs_sb")
                        nc.vector.tensor_copy(out=rs_sb, in_=rs_ps)
                        red_ps = ps_rs.tile([128, 512], dt.float32, tag="rs")
                        nc.tensor.matmul(
                            red_ps, lhsT=ones_full, rhs=rs_sb,
                            start=True, stop=True)
                        recip = small.tile([128, 512], dt.float32, tag="recip")
                        nc.vector.reciprocal(out=recip, in_=red_ps)
                        nc.vector.tensor_mul(
                            out=ot_sb[:, h, c * 512:(c + 1) * 512],
                            in0=ot_ps, in1=recip)

                # ---- output projection (partial) for batch b ----
                for nn in range(T // 128):  # 16 token tiles
                    o_sb = opool.tile([128, E], dt.bfloat16, tag="o")
                    for oc in range(E // 512):  # 4 output chunks
                        ps = ps_mm.tile([128, 512], dt.float32, tag="mm")
                        for j in range(HPC):
                            nc.tensor.matmul(
                                ps,
                                lhsT=ot_sb[:, j, nn * 128:(nn + 1) * 128],
                                rhs=w2_sb[:, j, oc * 512:(oc + 1) * 512],
                                start=(j == 0), stop=(j == HPC - 1),
                            )
                        if nn % 2 == 0:
                            nc.vector.tensor_copy(
                                out=o_sb[:, oc * 512:(oc + 1) * 512], in_=ps)
                        else:
                            nc.scalar.copy(
                                out=o_sb[:, oc * 512:(oc + 1) * 512], in_=ps)
                    if "outdma" not in ablate:
                        odma.dma_start(
                            out=out[b * T + nn * 128:b * T + (nn + 1) * 128, :],
                            in_=o_sb)


def prep_inputs(x: np.ndarray, w1: np.ndarray, w2: np.ndarray):
    """Host-side shard + transpose + bf16 cast. Returns in_maps for 8 cores."""
    # pretile x: chunk ci of 512 tokens -> [128 part, 16 e-tiles, 512 tok]
    xb = x.reshape(N // 512, 512, ET, 128).transpose(0, 3, 2, 1)
    xf = np.ascontiguousarray(xb).astype(BF16)
    w1r = w1.reshape(3, NH, D, E)
    in_maps = []
    for m in range(NCORES):
        hs = slice(HPC * m, HPC * (m + 1))
        wq = np.ascontiguousarray(w1r[0, hs].reshape(F, E).T).astype(BF16)
        wk = np.ascontiguousarray(w1r[1, hs].reshape(F, E).T).astype(BF16)
        wv = np.ascontiguousarray(w1r[2, hs].reshape(F, E).T).astype(BF16)
        w2s = np.ascontiguousarray(w2[:, F * m:F * (m + 1)].T).astype(BF16)
        in_maps.append({"xT": xf, "wqT": wq, "wkT": wk, "wvT": wv, "w2sT": w2s})
    return in_maps


def run(x, w1, w2, trace=False):
    from concourse import bass_utils

    nc = build_nc()
    in_maps = prep_inputs(np.asarray(x), np.asarray(w1), np.asarray(w2))
    res = bass_utils.run_bass_kernel_spmd(
        nc, in_maps, core_ids=list(range(NCORES)), trace=trace)
    acc = np.zeros((N, E), np.float32)
    for r in res.results:
        acc += r["out"]
    return acc.reshape(B, T, E), res


def kernel(x, w1, w2):
    out, _ = run(x, w1, w2, trace=False)
    return out
